# revision 36
# baseline (speedup 1.0000x reference)
"""Trainium2 Bass kernel for nn_ChaosSSMCore (selective diag-SSM).

Reference computation per (b, t):
    z, s, u, g = x @ {W_delta, W_select, W_in, W_gate}^T
    delta  = softplus(z)
    decay  = exp(-delta * exp(log_a))
    update = delta * sigmoid(s) * u
    states = scan: st = decay_t * st_{t-1} + update_t    (per (b, d) lane)
    out    = (states * silu(g)) @ W_out^T

Device mapping (8 cores, batch-sharded: 16 batches/core):
  * Host casts x to fp16; DMA hardware-transpose loads x^T [d, t] into SBUF
    so d (the contraction dim) is on partitions.
  * 4 input projections as fp16 matmuls (W^T stationary, x^T moving),
    PSUM results in [e, t] layout -> time on the free axis for the scan.
  * ScalarE uses ONE activation-table set (silu_and_others: tanh + silu) for
    the whole kernel -- softplus/sigmoid/exp are not all available in any
    single ACT table set and per-chunk set swaps cost ~2.7us each.
      tanh(z/2) -> decay = sigmoid(-z) = 0.5 - 0.5*tanh(z/2)   [log_a == 0]
      tanh(s/2) -> sigmoid(s) = 0.5 + 0.5*tanh(s/2)
      silu(g) native.
  * softplus is reconstructed from decay (exact identity, deg-2 fit of
    -ln(1-g), |err| < 5.1e-3):
      softplus(z) = relu(z) - ln(1 - min(decay, 1-decay))
  * VectorE: affines, update muls, tensor_tensor_scan (the recurrence).
  * GPSIMD: y = states * silu(g) (offloads the Vector engine).
  * Output projection uses y-blocks as the stationary operand so the result
    lands in PSUM already in natural [t, e'] layout; ScalarE copies it to
    SBUF fp16 and it is DMA'd out. Host upcasts to fp32.

log_a != 0 (never produced by setup_inputs, which inits log_a = zeros) falls
back to an exact numpy implementation since decay-via-sigmoid needs a == 1.
"""

import sys

for _p in ("/opt/trn_rl_repo", "/opt/pypackages"):
    if _p not in sys.path:
        sys.path.insert(0, _p)

import numpy as np

B, T, D = 128, 2048, 256
N_CORES = 8
NB = B // N_CORES          # batches per core
P = 128                    # SBUF partitions
CHUNK = 512                # tokens per pipeline chunk (1 PSUM bank fp32)
NCHUNK = T // CHUNK
KT = D // P                # contraction k-tiles (2)
MT = D // P                # output e-tiles (2)

PZ, PS, PU, PG, PO = 0, 1, 2, 3, 4   # weight slots: delta, select, in, gate, out

# deg-2 fit of -ln(1-g) ~= (g + SP_ADD)*g*SP_SCALE on g in [0, 0.5]
# (|err| < 5.1e-3 on delta). SP_SCALE is folded into host-scaled W_delta
# (z' = z/SP_SCALE) and the sigmoid affine (delta only ever feeds
# t1 = delta * sigmoid(s)), so the device computes
#   delta/SP_SCALE = relu(z') + (g + SP_ADD)*g,  g = min(dec, 1-dec)
SP_ADD = 0.9823220085291321
SP_SCALE = 0.9329222230329761


def build_bass(nb=NB):
    from contextlib import ExitStack

    import concourse.bacc as bacc
    import concourse.mybir as mybir
    import concourse.tile as tile

    f16 = mybir.dt.float16
    f32 = mybir.dt.float32
    ALU = mybir.AluOpType
    ACT = mybir.ActivationFunctionType

    nc = bacc.Bacc("TRN2", target_bir_lowering=False)

    ntok = nb * T
    # x arrives host-transposed: [batch, d, t] so the kernel loads x^T tiles
    # (d on partitions) with plain contiguous DMA.
    x_t = nc.dram_tensor("x", [nb, D, T], f16, kind="ExternalInput").ap()
    w_t = nc.dram_tensor("w", [P, 5, KT, D], f16, kind="ExternalInput").ap()
    out_t = nc.dram_tensor("out", [ntok, D], f16, kind="ExternalOutput").ap()

    with tile.TileContext(nc) as tc:
        with ExitStack() as ctx:
            singles = ctx.enter_context(tc.tile_pool(name="singles", bufs=1))
            xt_pool = ctx.enter_context(tc.tile_pool(name="xtp", bufs=4))
            sb = ctx.enter_context(tc.tile_pool(name="sb", bufs=4))
            osb_pool = ctx.enter_context(tc.tile_pool(name="osb", bufs=8))
            psum = ctx.enter_context(tc.tile_pool(name="psum", bufs=1, space="PSUM"))

            w_sb = singles.tile([P, 5, KT, D], f16)
            nc.scalar.dma_start(out=w_sb, in_=w_t)

            for b in range(nb):
                prev_states = None
                for c in range(NCHUNK):
                    row0 = b * T + c * CHUNK

                    # ---- load x^T tiles (host pre-transposed) ----
                    xt = [
                        xt_pool.tile([P, CHUNK], f16, tag=f"xt{k}", name=f"xt{k}")
                        for k in range(KT)
                    ]
                    for k in range(KT):
                        nc.sync.dma_start(
                            out=xt[k],
                            in_=x_t[
                                b,
                                k * P : (k + 1) * P,
                                c * CHUNK : (c + 1) * CHUNK,
                            ],
                        )

                    # ---- projections: psum[e_m, t] (z, s, u, g) ----
                    # one 2-bank psum tile per projection (both e-tiles) so
                    # downstream elementwise ops run FD=1024 instructions
                    pp = {}
                    for pi in (PZ, PS, PU, PG):
                        ps = psum.tile(
                            [P, MT, CHUNK], f32, tag="pp", bufs=3, name=f"pp{pi}"
                        )
                        for m in range(MT):
                            for k in range(KT):
                                nc.tensor.matmul(
                                    ps[:, m, :],
                                    w_sb[:, pi, k, m * P : (m + 1) * P],
                                    xt[k],
                                    start=(k == 0),
                                    stop=(k == KT - 1),
                                )
                        pp[pi] = ps

                    # ---- ScalarE (single table set: tanh + silu + relu) ----
                    tz = sb.tile([P, MT, CHUNK], f16, tag="tz")
                    tsl = sb.tile([P, MT, CHUNK], f16, tag="tsl")
                    gs = sb.tile([P, MT, CHUNK], f16, tag="gs")
                    rz = sb.tile([P, MT, CHUNK], f16, tag="rz")
                    # pp[PZ] holds z' = z/SP_SCALE (host-scaled W_delta); the
                    # tanh scale compensates so tz = tanh(z/2) exactly.
                    nc.scalar.activation(
                        out=tz, in_=pp[PZ], func=ACT.Tanh, scale=0.5 * SP_SCALE
                    )
                    nc.scalar.activation(out=rz, in_=pp[PZ], func=ACT.Relu)
                    nc.scalar.activation(out=tsl, in_=pp[PS], func=ACT.Tanh, scale=0.5)
                    nc.scalar.activation(out=gs, in_=pp[PG], func=ACT.Silu)
                    u16 = sb.tile([P, MT, CHUNK], f16, tag="u16")
                    nc.scalar.activation(out=u16, in_=pp[PU], func=ACT.Copy)

                    # ---- VectorE: decay, softplus, update, scan ----
                    dec = sb.tile([P, MT, CHUNK], f16, tag="dec")
                    inv = sb.tile([P, MT, CHUNK], f16, tag="inv")
                    gq = sb.tile([P, MT, CHUNK], f16, tag="gq")
                    pb = sb.tile([P, MT, CHUNK], f16, tag="pb")
                    delta = sb.tile([P, MT, CHUNK], f16, tag="delta")
                    t1 = sb.tile([P, MT, CHUNK], f16, tag="t1")
                    upd = sb.tile([P, MT, CHUNK], f16, tag="upd")
                    states = sb.tile([P, MT, CHUNK], f16, tag="states")
                    # decay = 0.5 - 0.5*tanh(z/2) = sigmoid(-z)
                    nc.vector.tensor_scalar(
                        out=dec, in0=tz, scalar1=-1.0, scalar2=-0.5,
                        op0=ALU.add, op1=ALU.mult,
                    )
                    # 1 - dec = 0.5 + 0.5*tanh(z/2)
                    nc.vector.tensor_scalar(
                        out=inv, in0=tz, scalar1=1.0, scalar2=0.5,
                        op0=ALU.add, op1=ALU.mult,
                    )
                    # softplus(z)/SP_SCALE = rz + (g + SP_ADD)*g, g=min(dec,1-dec)
                    nc.vector.tensor_tensor(out=gq, in0=dec, in1=inv, op=ALU.min)
                    nc.vector.scalar_tensor_tensor(
                        out=pb, in0=gq, scalar=SP_ADD, in1=gq,
                        op0=ALU.add, op1=ALU.mult,
                    )
                    nc.gpsimd.tensor_add(delta, pb, rz)
                    # t1 = (tanh(s/2)+1) * delta'; the 0.5*SP_SCALE of
                    # sigmoid(s)*SP_SCALE is folded into host-scaled W_in.
                    nc.vector.scalar_tensor_tensor(
                        out=t1, in0=tsl, scalar=1.0, in1=delta,
                        op0=ALU.add, op1=ALU.mult,
                    )
                    nc.vector.tensor_mul(upd, t1, u16)
                    for m in range(MT):
                        init = (
                            0.0
                            if prev_states is None
                            else prev_states[:, m, CHUNK - 1 : CHUNK]
                        )
                        nc.vector.tensor_tensor_scan(
                            out=states[:, m, :],
                            data0=dec[:, m, :],
                            data1=upd[:, m, :],
                            initial=init,
                            op0=ALU.mult,
                            op1=ALU.add,
                        )
                    prev_states = states

                    # ---- GPSIMD: y = states * silu(g) ----
                    y = sb.tile([P, MT, CHUNK], f16, tag="y")
                    nc.gpsimd.tensor_mul(y, states, gs)

                    # ---- out projection: y blocks stationary -> [t, e'] ----
                    for pair in range(CHUNK // P // 2):
                        po = psum.tile([P, 2, D], f32, tag="po", bufs=2)
                        for tj in range(2):
                            tt = pair * 2 + tj
                            for k in range(KT):
                                nc.tensor.matmul(
                                    po[:, tj, :],
                                    y[:, k, tt * P : (tt + 1) * P],
                                    w_sb[:, PO, k, :],
                                    start=(k == 0),
                                    stop=(k == KT - 1),
                                )
                        osb = osb_pool.tile([P, 2, D], f16, tag="osb")
                        nc.scalar.activation(out=osb, in_=po, func=ACT.Copy)
                        nc.sync.dma_start(
                            out=out_t[
                                row0 + pair * 2 * P : row0 + (pair + 1) * 2 * P, :
                            ].rearrange("(j p) d -> p j d", p=P),
                            in_=osb,
                        )
    nc.compile()
    return nc


def _pack_weight(w):
    # lhsT layout: [d_within_k (partition), k, e] with lhsT[dd, k, e] = W[e, 128k+dd]
    return (
        np.ascontiguousarray(np.asarray(w, np.float32).T)
        .reshape(KT, P, D)
        .transpose(1, 0, 2)
        .astype(np.float16)
    )


def prepare_inputs(x, W_in, W_select, W_gate, W_out, W_delta, log_a):
    x16 = (
        np.ascontiguousarray(np.asarray(x, np.float32))
        .astype(np.float16)
        .reshape(N_CORES, NB, T, D)
        .transpose(0, 1, 3, 2)  # -> [core, batch, d, t]
    )
    x16 = np.ascontiguousarray(x16)
    # W_delta scaled by 1/SP_SCALE (softplus poly leading-coeff fold);
    # W_in scaled by 0.5*SP_SCALE (sigmoid affine + that fold's inverse:
    # update = delta'*(1+tanh(s/2)) * u' with u' = u*0.5*SP_SCALE)
    w_delta_scaled = np.asarray(W_delta, np.float32) / SP_SCALE
    w_in_scaled = np.asarray(W_in, np.float32) * (0.5 * SP_SCALE)
    w_pack = np.ascontiguousarray(
        np.stack(
            [
                _pack_weight(w)
                for w in (w_delta_scaled, W_select, w_in_scaled, W_gate, W_out)
            ],
            axis=1,
        )
    )  # [P, 5, KT, D]
    return [{"x": x16[c], "w": w_pack} for c in range(N_CORES)]


def _numpy_fallback(x, W_in, W_select, W_gate, W_out, W_delta, log_a):
    # exact reference math; only used when log_a != 0 (setup_inputs never does)
    x = np.asarray(x, np.float32)
    z = x @ np.asarray(W_delta, np.float32).T
    delta = np.logaddexp(0.0, z)
    decay = np.exp(-delta * np.exp(np.asarray(log_a, np.float32)))
    u = x @ np.asarray(W_in, np.float32).T
    s = x @ np.asarray(W_select, np.float32).T
    upd = delta * (1.0 / (1.0 + np.exp(-s))) * u
    states = np.empty_like(upd)
    st = np.zeros((x.shape[0], x.shape[2]), np.float32)
    for t in range(x.shape[1]):
        st = decay[:, t] * st + upd[:, t]
        states[:, t] = st
    g = x @ np.asarray(W_gate, np.float32).T
    y = states * (g / (1.0 + np.exp(-g)))
    return y @ np.asarray(W_out, np.float32).T


_CACHE = {}


def run_on_hw(inputs, trace=False):
    from concourse.bass_utils import run_bass_kernel_spmd

    if "nc" not in _CACHE:
        _CACHE["nc"] = build_bass()
    nc = _CACHE["nc"]
    in_maps = prepare_inputs(**inputs)
    res = run_bass_kernel_spmd(nc, in_maps, core_ids=list(range(N_CORES)), trace=trace)
    out = (
        np.stack([res.results[c]["out"] for c in range(N_CORES)])
        .reshape(B, T, D)
        .astype(np.float32)
    )
    return out, res


def kernel(x, W_in, W_select, W_gate, W_out, W_delta, log_a):
    inputs = dict(
        x=x,
        W_in=W_in,
        W_select=W_select,
        W_gate=W_gate,
        W_out=W_out,
        W_delta=W_delta,
        log_a=log_a,
    )
    if not np.allclose(np.asarray(log_a, np.float32), 0.0):
        return _numpy_fallback(**inputs)
    out, _ = run_on_hw(inputs)
    return out
